# revision 35
# baseline (speedup 1.0000x reference)
"""GATv2 layer kernel for Trainium2, sharded across 8 NeuronCores.

Computation (reference):
    Wh = h @ W.T                       [N, F]
    s1 = Wh @ a1, s2 = Wh @ a2         [N]
    e  = leaky_relu(s1[:,None] + s2[None,:], 0.2)
    attention = softmax(e * adj, dim=1)
    out = attention @ Wh               [N, F]

Sharding: rows (destination nodes) split across 8 cores, 1024 rows each.
Each core gets its adj row-block plus replicated h/W/a, computes its
1024x128 output block; host concatenates.

adj is 0/1-valued so the host casts it to bf16 losslessly; this halves the
HBM stream AND enables the DMA xbar transpose (2-byte dtypes only), which
delivers adj^T tiles [c, r] directly. The whole pipeline then runs in
transposed layout and the PE never transposes anything:

    per column-chunk ci (128 source nodes x all 1024 own rows):
      DMA : adjT = transpose-DMA adj[:, ci-block]          [128c, 1024r]
      ACT : L = Prelu(SIbc + s2_col[ci], 0.2)  (bias = per-partition s2)
            (or a fused DVE stt pair, load-balanced via PHI)
      DVE : T = L * adjT        (bf16 2x mode)
      ACT : P = Exp(T)          (bf16, sbuf->sbuf, 2-chunk batches)
      PE  : acc[t] += P[:, t-slice].T @ [Wh | 1]   for the 8 row-tiles
    finalize: out_rows[t] = acc[t][:, :128] / acc[t][:, 128]

Softmax runs without max subtraction: scores are O(6) so exp stays in
fp32 range; matches the reference up to fp rounding.
"""
import sys

for _p in ("/opt/trn_rl_repo", "/root/.axon_site/_ro/trn_rl_repo"):
    if _p not in sys.path:
        sys.path.insert(0, _p)

import numpy as np
import ml_dtypes
from contextlib import ExitStack

from concourse import bacc, tile, mybir
from concourse.bass_utils import run_bass_kernel_spmd

f32 = mybir.dt.float32
bf16 = mybir.dt.bfloat16
AL = mybir.AluOpType
AF = mybir.ActivationFunctionType

N = 8192
F = 128
NCORES = 8
RPC = N // NCORES          # rows per core = 1024
RT = RPC // 128            # row tiles per core = 8
NCI = N // 128             # column chunks = 64
PHI = 33                   # of 64 chunks routed to the DVE leaky path
NEG_SLOPE = 0.2

_CACHE = {}


def _build():
    nc = bacc.Bacc("TRN2", target_bir_lowering=False)

    adj_ext = nc.declare_dram_parameter("adjT", [N, RPC], bf16, isOutput=False)
    hT_ext = nc.declare_dram_parameter("hT", [F, N], f32, isOutput=False)
    hTloc_ext = nc.declare_dram_parameter("hT_loc", [F, RPC], f32, isOutput=False)
    wt_ext = nc.declare_dram_parameter("wt", [F, F], f32, isOutput=False)  # W^T
    w_ext = nc.declare_dram_parameter("w", [F, F], f32, isOutput=False)    # W
    a1_ext = nc.declare_dram_parameter("a1", [F, 1], f32, isOutput=False)
    a2_ext = nc.declare_dram_parameter("a2", [F, 1], f32, isOutput=False)
    out_ext = nc.declare_dram_parameter("out", [RPC, F], f32, isOutput=True)

    with tile.TileContext(nc) as tc, ExitStack() as ctx:
        const = ctx.enter_context(tc.tile_pool(name="const", bufs=1))
        setup = ctx.enter_context(tc.tile_pool(name="setup", bufs=3))
        psum = ctx.enter_context(tc.tile_pool(name="psum", bufs=8, space="PSUM"))
        adj_pool = ctx.enter_context(tc.tile_pool(name="adjp", bufs=10))
        work = ctx.enter_context(tc.tile_pool(name="work", bufs=6))
        pexp = ctx.enter_context(tc.tile_pool(name="pexp", bufs=4))
        outp = ctx.enter_context(tc.tile_pool(name="outp", bufs=2))

        wt_sb = const.tile([F, F], f32)
        nc.sync.dma_start(out=wt_sb, in_=wt_ext[:, :])
        w_sb = const.tile([F, F], f32)
        nc.sync.dma_start(out=w_sb, in_=w_ext[:, :])
        a1_sb = const.tile([F, 1], f32)
        nc.sync.dma_start(out=a1_sb, in_=a1_ext[:, :])
        a2_sb = const.tile([F, 1], f32)
        nc.sync.dma_start(out=a2_sb, in_=a2_ext[:, :])
        ones_row = const.tile([1, 128], f32)
        nc.vector.memset(ones_row, 1.0)

        # persistent tensors
        whext_t = [const.tile([128, 8, F + 1], bf16, name=f"whext{_}")
                   for _ in range(8)]
        sj_cols = [const.tile([128, 8], f32, name=f"sjc{_}")
                   for _ in range(8)]            # s2, column layout, per k
        sibc = const.tile([128, RPC], f32)        # s1 own rows, bcast over parts
        sibc_bf = const.tile([128, RPC], bf16)    # bf16 twin for the DVE path
        for jj in range(8):
            nc.vector.memset(whext_t[jj][:, :, F:F + 1], 1.0)

        # w1 = W^T a1, w2 = W^T a2 ; wt2 = [W^T | w2]
        ps_w = psum.tile([128, 512], f32, tag="acc")
        nc.tensor.matmul(ps_w[:, 0:1], lhsT=w_sb, rhs=a1_sb, start=True, stop=True)
        nc.tensor.matmul(ps_w[:, 1:2], lhsT=w_sb, rhs=a2_sb, start=True, stop=True)
        w1c = const.tile([128, 1], f32)
        nc.vector.tensor_copy(out=w1c, in_=ps_w[:, 0:1])
        wt2_sb = const.tile([F, F + 1], f32)
        nc.vector.tensor_copy(out=wt2_sb[:, 0:F], in_=wt_sb)
        nc.vector.tensor_copy(out=wt2_sb[:, F:F + 1], in_=ps_w[:, 1:2])

        # s1 own rows -> free layout -> broadcast across partitions
        si_sb = const.tile([1, RPC], f32)
        for kk in range(RPC // 512):
            hTlc = setup.tile([128, 512], f32, tag="hTlc", name=f"hTlc{kk}")
            nc.sync.dma_start(out=hTlc,
                              in_=hTloc_ext[:, 512 * kk:512 * kk + 512])
            ps_si = psum.tile([128, 512], f32, tag="acc", name=f"psi{kk}")
            nc.tensor.matmul(ps_si[0:1, 0:512], lhsT=w1c, rhs=hTlc,
                             start=True, stop=True)
            nc.vector.tensor_copy(out=si_sb[0:1, 512 * kk:512 * kk + 512],
                                  in_=ps_si[0:1, 0:512])
        for kk in range(RPC // 512):
            ps_sib = psum.tile([128, 512], f32, tag="acc", name=f"psib{kk}")
            nc.tensor.matmul(ps_sib[:, 0:512], lhsT=ones_row,
                             rhs=si_sb[0:1, 512 * kk:512 * kk + 512],
                             start=True, stop=True)
            nc.scalar.copy(out=sibc[:, 512 * kk:512 * kk + 512],
                           in_=ps_sib[:, 0:512])
            nc.vector.tensor_copy(out=sibc_bf[:, 512 * kk:512 * kk + 512],
                                  in_=ps_sib[:, 0:512])

        # stream hT chunks: whext tiles (Wh | 1) and s2 columns via [W^T | w2]
        def emit_setup_k(k):
            hTc = setup.tile([128, 1024], f32, tag="hTc", name=f"hTc{k}")
            nc.sync.dma_start(out=hTc, in_=hT_ext[:, 1024 * k:1024 * k + 1024])
            for m in range(8):
                ci = 8 * k + m
                ps2 = psum.tile([128, 512], f32, tag="acc", name=f"pwh{ci}")
                nc.tensor.matmul(ps2[:, 0:F + 1],
                                 lhsT=hTc[:, 128 * m:128 * m + 128],
                                 rhs=wt2_sb, start=True, stop=True)
                nc.vector.tensor_copy(out=whext_t[k][:, m, 0:F],
                                      in_=ps2[:, 0:F])
                nc.vector.tensor_copy(out=sj_cols[k][:, m:m + 1],
                                      in_=ps2[:, F:F + 1])

        # main chunk: 128 source nodes x all own rows
        def emit_main_ci(ci, accs, pair_buf):
            adjT = adj_pool.tile([128, RPC], bf16, tag="adjT", name=f"adjT{ci}")
            nc.sync.dma_start(out=adjT,
                              in_=adj_ext[128 * ci:128 * ci + 128, :])
            q = ci % 4
            if (((ci + 1) * PHI) // NCI) > ((ci * PHI) // NCI):
                # DVE path: 4x-ts add, 2x-tt mask, 4x-ts scale, 2x-tt max
                u1 = work.tile([128, RPC], bf16, tag="u1", name=f"u1_{ci}")
                nc.vector.tensor_scalar(
                    out=u1, in0=sibc_bf,
                    scalar1=sj_cols[ci // 8][:, ci % 8:ci % 8 + 1],
                    scalar2=None, op0=AL.add)
                T0 = work.tile([128, RPC], bf16, tag="T0", name=f"T0_{ci}")
                nc.vector.tensor_tensor(out=T0, in0=u1, in1=adjT, op=AL.mult)
                u2 = work.tile([128, RPC], bf16, tag="u2", name=f"u2_{ci}")
                nc.vector.tensor_scalar(out=u2, in0=T0, scalar1=NEG_SLOPE,
                                        scalar2=None, op0=AL.mult)
                nc.vector.tensor_tensor(out=pair_buf[:, RPC * q:RPC * q + RPC],
                                        in0=u2, in1=T0, op=AL.max)
            else:
                L = work.tile([128, RPC], bf16, tag="L", name=f"L_{ci}")
                nc.scalar.activation(out=L, in_=sibc, func=AF.Prelu,
                                     bias=sj_cols[ci // 8][:, ci % 8:ci % 8 + 1],
                                     alpha=NEG_SLOPE)
                nc.vector.tensor_tensor(out=pair_buf[:, RPC * q:RPC * q + RPC],
                                        in0=L, in1=adjT, op=AL.mult)
            if q == 3:
                P2 = pexp.tile([128, 4 * RPC], bf16, tag="P", name=f"P{ci}")
                nc.scalar.activation(out=P2, in_=pair_buf, func=AF.Exp)
                for h in range(4):
                    cih = ci - 3 + h
                    for t in range(RT):
                        nc.tensor.matmul(
                            accs[t],
                            lhsT=P2[:, RPC * h + 128 * t:RPC * h + 128 * t + 128],
                            rhs=whext_t[cih // 8][:, cih % 8, :],
                            start=(cih == 0 and t % 2 == 0),
                            stop=(cih == NCI - 1),
                            skip_group_check=True)

        acc_banks = [psum.tile([128, 512], f32, tag="acc", name=f"accb{b}")
                     for b in range(RT // 2)]
        accs = [acc_banks[t // 2][:, 256 * (t % 2):256 * (t % 2) + F + 1]
                for t in range(RT)]

        def emit_main(ci_iter, pair):
            if ci_iter % 4 == 0:
                pair = work.tile([128, 4 * RPC], bf16, tag="T",
                                 name=f"Tp{ci_iter}")
            emit_main_ci(ci_iter, accs, pair)
            return pair

        pair = None
        emit_setup_k(0)
        emit_setup_k(1)
        ci_iter = 0
        for k in range(2, 8):
            emit_setup_k(k)
            while ci_iter < 8 * (k - 1):
                pair = emit_main(ci_iter, pair)
                ci_iter += 1
        while ci_iter < NCI:
            pair = emit_main(ci_iter, pair)
            ci_iter += 1

        for t in range(RT):
            rinv = outp.tile([128, 1], f32, tag="rinv", name=f"rinv{t}")
            nc.vector.reciprocal(rinv, accs[t][:, F:F + 1])
            o_t = outp.tile([128, F], f32, tag="o", name=f"o{t}")
            nc.vector.tensor_scalar(out=o_t, in0=accs[t][:, 0:F],
                                    scalar1=rinv[:, 0:1], scalar2=None,
                                    op0=AL.mult)
            nc.sync.dma_start(out=out_ext[128 * t:128 * t + 128, :], in_=o_t)

    nc.compile()
    return nc


def _get_nc():
    if "nc" not in _CACHE:
        _CACHE["nc"] = _build()
    return _CACHE["nc"]


def kernel(h, adj, W, a, _trace=False, _trace_kwargs=None):
    h = np.ascontiguousarray(np.asarray(h, dtype=np.float32))
    adj = np.asarray(adj, dtype=np.float32)
    W = np.asarray(W, dtype=np.float32)
    a = np.asarray(a, dtype=np.float32)

    wt = np.ascontiguousarray(W.T)                    # [fi, fo]
    a1c = np.ascontiguousarray(a[0, :F].reshape(F, 1))
    a2c = np.ascontiguousarray(a[0, F:].reshape(F, 1))
    hT = np.ascontiguousarray(h.T)                    # [fi, n]
    adjT_bf = adj.astype(ml_dtypes.bfloat16).T        # 0/1 values: lossless

    nc = _get_nc()
    in_maps = []
    for c in range(NCORES):
        r0 = c * RPC
        in_maps.append({
            "adjT": np.ascontiguousarray(adjT_bf[:, r0:r0 + RPC]),
            "hT": hT,
            "hT_loc": np.ascontiguousarray(hT[:, r0:r0 + RPC]),
            "wt": wt,
            "w": W,
            "a1": a1c,
            "a2": a2c,
        })
    kw = {}
    if _trace:
        kw["trace"] = True
        kw.update(_trace_kwargs or {})
    res = run_bass_kernel_spmd(nc, in_maps, core_ids=list(range(NCORES)), **kw)
    out = np.concatenate([res.results[c]["out"] for c in range(NCORES)], axis=0)
    if _trace:
        return out, res
    return out


# revision 36
# speedup vs baseline: 1.0278x; 1.0278x over previous
"""GATv2 layer kernel for Trainium2, sharded across 8 NeuronCores.

Computation (reference):
    Wh = h @ W.T                       [N, F]
    s1 = Wh @ a1, s2 = Wh @ a2         [N]
    e  = leaky_relu(s1[:,None] + s2[None,:], 0.2)
    attention = softmax(e * adj, dim=1)
    out = attention @ Wh               [N, F]

Sharding: rows (destination nodes) split across 8 cores, 1024 rows each.
Each core gets its adj row-block plus replicated h/W/a, computes its
1024x128 output block; host concatenates.

adj is 0/1-valued so the host casts it to bf16 losslessly; this halves the
HBM stream AND enables the DMA xbar transpose (2-byte dtypes only), which
delivers adj^T tiles [c, r] directly. The whole pipeline then runs in
transposed layout and the PE never transposes anything:

    per column-chunk ci (128 source nodes x all 1024 own rows):
      DMA : adjT = transpose-DMA adj[:, ci-block]          [128c, 1024r]
      ACT : L = Prelu(SIbc + s2_col[ci], 0.2)  (bias = per-partition s2)
            (or a fused DVE stt pair, load-balanced via PHI)
      DVE : T = L * adjT        (bf16 2x mode)
      ACT : P = Exp(T)          (bf16, sbuf->sbuf, 2-chunk batches)
      PE  : acc[t] += P[:, t-slice].T @ [Wh | 1]   for the 8 row-tiles
    finalize: out_rows[t] = acc[t][:, :128] / acc[t][:, 128]

Softmax runs without max subtraction: scores are O(6) so exp stays in
fp32 range; matches the reference up to fp rounding.
"""
import sys

for _p in ("/opt/trn_rl_repo", "/root/.axon_site/_ro/trn_rl_repo"):
    if _p not in sys.path:
        sys.path.insert(0, _p)

import numpy as np
import ml_dtypes
from contextlib import ExitStack

from concourse import bacc, tile, mybir
from concourse.bass_utils import run_bass_kernel_spmd

f32 = mybir.dt.float32
bf16 = mybir.dt.bfloat16
AL = mybir.AluOpType
AF = mybir.ActivationFunctionType

N = 8192
F = 128
NCORES = 8
RPC = N // NCORES          # rows per core = 1024
RT = RPC // 128            # row tiles per core = 8
NCI = N // 128             # column chunks = 64
PHI = 30                   # of 64 chunks routed to the DVE leaky path
NEG_SLOPE = 0.2

_CACHE = {}


def _build():
    nc = bacc.Bacc("TRN2", target_bir_lowering=False)

    adj_ext = nc.declare_dram_parameter("adjT", [N, RPC], bf16, isOutput=False)
    hT_ext = nc.declare_dram_parameter("hT", [F, N], f32, isOutput=False)
    hTloc_ext = nc.declare_dram_parameter("hT_loc", [F, RPC], f32, isOutput=False)
    wt_ext = nc.declare_dram_parameter("wt", [F, F], f32, isOutput=False)  # W^T
    w_ext = nc.declare_dram_parameter("w", [F, F], f32, isOutput=False)    # W
    a1_ext = nc.declare_dram_parameter("a1", [F, 1], f32, isOutput=False)
    a2_ext = nc.declare_dram_parameter("a2", [F, 1], f32, isOutput=False)
    out_ext = nc.declare_dram_parameter("out", [RPC, F], f32, isOutput=True)

    with tile.TileContext(nc) as tc, ExitStack() as ctx:
        const = ctx.enter_context(tc.tile_pool(name="const", bufs=1))
        setup = ctx.enter_context(tc.tile_pool(name="setup", bufs=3))
        psum = ctx.enter_context(tc.tile_pool(name="psum", bufs=8, space="PSUM"))
        adj_pool = ctx.enter_context(tc.tile_pool(name="adjp", bufs=8))
        work = ctx.enter_context(tc.tile_pool(name="work", bufs=6))
        pexp = ctx.enter_context(tc.tile_pool(name="pexp", bufs=4))
        outp = ctx.enter_context(tc.tile_pool(name="outp", bufs=2))

        wt_sb = const.tile([F, F], f32)
        nc.sync.dma_start(out=wt_sb, in_=wt_ext[:, :])
        w_sb = const.tile([F, F], f32)
        nc.sync.dma_start(out=w_sb, in_=w_ext[:, :])
        a1_sb = const.tile([F, 1], f32)
        nc.sync.dma_start(out=a1_sb, in_=a1_ext[:, :])
        a2_sb = const.tile([F, 1], f32)
        nc.sync.dma_start(out=a2_sb, in_=a2_ext[:, :])
        ones_row = const.tile([1, 128], f32)
        nc.vector.memset(ones_row, 1.0)

        # persistent tensors
        whext_t = [const.tile([128, 8, F + 1], bf16, name=f"whext{_}")
                   for _ in range(8)]
        sj_cols = [const.tile([128, 8], f32, name=f"sjc{_}")
                   for _ in range(8)]            # s2, column layout, per k
        sibc = const.tile([128, RPC], f32)        # s1 own rows, bcast over parts
        sibc_bf = const.tile([128, RPC], bf16)    # bf16 twin for the DVE path
        for jj in range(8):
            nc.vector.memset(whext_t[jj][:, :, F:F + 1], 1.0)

        # w1 = W^T a1, w2 = W^T a2 ; wt2 = [W^T | w2]
        ps_w = psum.tile([128, 512], f32, tag="acc")
        nc.tensor.matmul(ps_w[:, 0:1], lhsT=w_sb, rhs=a1_sb, start=True, stop=True)
        nc.tensor.matmul(ps_w[:, 1:2], lhsT=w_sb, rhs=a2_sb, start=True, stop=True)
        w1c = const.tile([128, 1], f32)
        nc.vector.tensor_copy(out=w1c, in_=ps_w[:, 0:1])
        wt2_sb = const.tile([F, F + 1], f32)
        nc.vector.tensor_copy(out=wt2_sb[:, 0:F], in_=wt_sb)
        nc.vector.tensor_copy(out=wt2_sb[:, F:F + 1], in_=ps_w[:, 1:2])

        # s1 own rows -> free layout -> broadcast across partitions
        si_sb = const.tile([1, RPC], f32)
        for kk in range(RPC // 512):
            hTlc = setup.tile([128, 512], f32, tag="hTlc", name=f"hTlc{kk}")
            nc.sync.dma_start(out=hTlc,
                              in_=hTloc_ext[:, 512 * kk:512 * kk + 512])
            ps_si = psum.tile([128, 512], f32, tag="acc", name=f"psi{kk}")
            nc.tensor.matmul(ps_si[0:1, 0:512], lhsT=w1c, rhs=hTlc,
                             start=True, stop=True)
            nc.vector.tensor_copy(out=si_sb[0:1, 512 * kk:512 * kk + 512],
                                  in_=ps_si[0:1, 0:512])
        for kk in range(RPC // 512):
            ps_sib = psum.tile([128, 512], f32, tag="acc", name=f"psib{kk}")
            nc.tensor.matmul(ps_sib[:, 0:512], lhsT=ones_row,
                             rhs=si_sb[0:1, 512 * kk:512 * kk + 512],
                             start=True, stop=True)
            nc.scalar.copy(out=sibc[:, 512 * kk:512 * kk + 512],
                           in_=ps_sib[:, 0:512])
            nc.vector.tensor_copy(out=sibc_bf[:, 512 * kk:512 * kk + 512],
                                  in_=ps_sib[:, 0:512])

        # stream hT chunks: whext tiles (Wh | 1) and s2 columns via [W^T | w2]
        def emit_setup_k(k):
            hTc = setup.tile([128, 1024], f32, tag="hTc", name=f"hTc{k}")
            nc.sync.dma_start(out=hTc, in_=hT_ext[:, 1024 * k:1024 * k + 1024])
            for m in range(8):
                ci = 8 * k + m
                ps2 = psum.tile([128, 512], f32, tag="acc", name=f"pwh{ci}")
                nc.tensor.matmul(ps2[:, 0:F + 1],
                                 lhsT=hTc[:, 128 * m:128 * m + 128],
                                 rhs=wt2_sb, start=True, stop=True)
                nc.vector.tensor_copy(out=whext_t[k][:, m, 0:F],
                                      in_=ps2[:, 0:F])
                nc.vector.tensor_copy(out=sj_cols[k][:, m:m + 1],
                                      in_=ps2[:, F:F + 1])

        # main chunk: 128 source nodes x all own rows
        def emit_main_ci(ci, accs, pair_buf):
            adjT = adj_pool.tile([128, RPC], bf16, tag="adjT", name=f"adjT{ci}")
            nc.sync.dma_start(out=adjT,
                              in_=adj_ext[128 * ci:128 * ci + 128, :])
            q = ci % 4
            if (((ci + 1) * PHI) // NCI) > ((ci * PHI) // NCI):
                # DVE path: 4x-ts add, 2x-tt mask, 4x-ts scale, 2x-tt max
                u1 = work.tile([128, RPC], bf16, tag="u1", name=f"u1_{ci}")
                nc.vector.tensor_scalar(
                    out=u1, in0=sibc_bf,
                    scalar1=sj_cols[ci // 8][:, ci % 8:ci % 8 + 1],
                    scalar2=None, op0=AL.add)
                T0 = work.tile([128, RPC], bf16, tag="T0", name=f"T0_{ci}")
                nc.vector.tensor_tensor(out=T0, in0=u1, in1=adjT, op=AL.mult)
                u2 = work.tile([128, RPC], bf16, tag="u2", name=f"u2_{ci}")
                nc.vector.tensor_scalar(out=u2, in0=T0, scalar1=NEG_SLOPE,
                                        scalar2=None, op0=AL.mult)
                nc.vector.tensor_tensor(out=pair_buf[:, RPC * q:RPC * q + RPC],
                                        in0=u2, in1=T0, op=AL.max)
            else:
                L = work.tile([128, RPC], bf16, tag="L", name=f"L_{ci}")
                nc.scalar.activation(out=L, in_=sibc, func=AF.Prelu,
                                     bias=sj_cols[ci // 8][:, ci % 8:ci % 8 + 1],
                                     alpha=NEG_SLOPE)
                nc.vector.tensor_tensor(out=pair_buf[:, RPC * q:RPC * q + RPC],
                                        in0=L, in1=adjT, op=AL.mult)
            if q == 3:
                P2 = pexp.tile([128, 4 * RPC], bf16, tag="P", name=f"P{ci}")
                nc.scalar.activation(out=P2, in_=pair_buf, func=AF.Exp)
                for h in range(4):
                    cih = ci - 3 + h
                    for t in range(RT):
                        nc.tensor.matmul(
                            accs[t],
                            lhsT=P2[:, RPC * h + 128 * t:RPC * h + 128 * t + 128],
                            rhs=whext_t[cih // 8][:, cih % 8, :],
                            start=(cih == 0 and t % 2 == 0),
                            stop=(cih == NCI - 1),
                            skip_group_check=True)

        acc_banks = [psum.tile([128, 512], f32, tag="acc", name=f"accb{b}")
                     for b in range(RT // 2)]
        accs = [acc_banks[t // 2][:, 256 * (t % 2):256 * (t % 2) + F + 1]
                for t in range(RT)]

        def emit_main(ci_iter, pair):
            if ci_iter % 4 == 0:
                pair = work.tile([128, 4 * RPC], bf16, tag="T",
                                 name=f"Tp{ci_iter}")
            emit_main_ci(ci_iter, accs, pair)
            return pair

        pair = None
        emit_setup_k(0)
        emit_setup_k(1)
        ci_iter = 0
        for k in range(2, 8):
            emit_setup_k(k)
            while ci_iter < 8 * (k - 1):
                pair = emit_main(ci_iter, pair)
                ci_iter += 1
        while ci_iter < NCI:
            pair = emit_main(ci_iter, pair)
            ci_iter += 1

        for t in range(RT):
            rinv = outp.tile([128, 1], f32, tag="rinv", name=f"rinv{t}")
            nc.vector.reciprocal(rinv, accs[t][:, F:F + 1])
            o_t = outp.tile([128, F], f32, tag="o", name=f"o{t}")
            nc.vector.tensor_scalar(out=o_t, in0=accs[t][:, 0:F],
                                    scalar1=rinv[:, 0:1], scalar2=None,
                                    op0=AL.mult)
            nc.sync.dma_start(out=out_ext[128 * t:128 * t + 128, :], in_=o_t)

    nc.compile()
    return nc


def _get_nc():
    if "nc" not in _CACHE:
        _CACHE["nc"] = _build()
    return _CACHE["nc"]


def kernel(h, adj, W, a, _trace=False, _trace_kwargs=None):
    h = np.ascontiguousarray(np.asarray(h, dtype=np.float32))
    adj = np.asarray(adj, dtype=np.float32)
    W = np.asarray(W, dtype=np.float32)
    a = np.asarray(a, dtype=np.float32)

    wt = np.ascontiguousarray(W.T)                    # [fi, fo]
    a1c = np.ascontiguousarray(a[0, :F].reshape(F, 1))
    a2c = np.ascontiguousarray(a[0, F:].reshape(F, 1))
    hT = np.ascontiguousarray(h.T)                    # [fi, n]
    adjT_bf = adj.astype(ml_dtypes.bfloat16).T        # 0/1 values: lossless

    nc = _get_nc()
    in_maps = []
    for c in range(NCORES):
        r0 = c * RPC
        in_maps.append({
            "adjT": np.ascontiguousarray(adjT_bf[:, r0:r0 + RPC]),
            "hT": hT,
            "hT_loc": np.ascontiguousarray(hT[:, r0:r0 + RPC]),
            "wt": wt,
            "w": W,
            "a1": a1c,
            "a2": a2c,
        })
    kw = {}
    if _trace:
        kw["trace"] = True
        kw.update(_trace_kwargs or {})
    res = run_bass_kernel_spmd(nc, in_maps, core_ids=list(range(NCORES)), **kw)
    out = np.concatenate([res.results[c]["out"] for c in range(NCORES)], axis=0)
    if _trace:
        return out, res
    return out


# revision 37
# speedup vs baseline: 1.0446x; 1.0163x over previous
"""GATv2 layer kernel for Trainium2, sharded across 8 NeuronCores.

Computation (reference):
    Wh = h @ W.T                       [N, F]
    s1 = Wh @ a1, s2 = Wh @ a2         [N]
    e  = leaky_relu(s1[:,None] + s2[None,:], 0.2)
    attention = softmax(e * adj, dim=1)
    out = attention @ Wh               [N, F]

Sharding: rows (destination nodes) split across 8 cores, 1024 rows each.
Each core gets its adj row-block plus replicated h/W/a, computes its
1024x128 output block; host concatenates.

adj is 0/1-valued so the host casts it to bf16 losslessly; this halves the
HBM stream AND enables the DMA xbar transpose (2-byte dtypes only), which
delivers adj^T tiles [c, r] directly. The whole pipeline then runs in
transposed layout and the PE never transposes anything:

    per column-chunk ci (128 source nodes x all 1024 own rows):
      DMA : adjT = transpose-DMA adj[:, ci-block]          [128c, 1024r]
      ACT : L = Prelu(SIbc + s2_col[ci], 0.2)  (bias = per-partition s2)
            (or a fused DVE stt pair, load-balanced via PHI)
      DVE : T = L * adjT        (bf16 2x mode)
      ACT : P = Exp(T)          (bf16, sbuf->sbuf, 2-chunk batches)
      PE  : acc[t] += P[:, t-slice].T @ [Wh | 1]   for the 8 row-tiles
    finalize: out_rows[t] = acc[t][:, :128] / acc[t][:, 128]

Softmax runs without max subtraction: scores are O(6) so exp stays in
fp32 range; matches the reference up to fp rounding.
"""
import sys

for _p in ("/opt/trn_rl_repo", "/root/.axon_site/_ro/trn_rl_repo"):
    if _p not in sys.path:
        sys.path.insert(0, _p)

import numpy as np
import ml_dtypes
from contextlib import ExitStack

from concourse import bacc, tile, mybir
from concourse.bass_utils import run_bass_kernel_spmd

f32 = mybir.dt.float32
bf16 = mybir.dt.bfloat16
AL = mybir.AluOpType
AF = mybir.ActivationFunctionType

N = 8192
F = 128
NCORES = 8
RPC = N // NCORES          # rows per core = 1024
RT = RPC // 128            # row tiles per core = 8
NCI = N // 128             # column chunks = 64
PHI = 30                   # of 64 chunks routed to the DVE leaky path
NEG_SLOPE = 0.2

_CACHE = {}


def _build():
    nc = bacc.Bacc("TRN2", target_bir_lowering=False)

    adj_ext = nc.declare_dram_parameter("adjT", [N, RPC], bf16, isOutput=False)
    hT_ext = nc.declare_dram_parameter("hT", [F, N], f32, isOutput=False)
    hTloc_ext = nc.declare_dram_parameter("hT_loc", [F, RPC], f32, isOutput=False)
    wt_ext = nc.declare_dram_parameter("wt", [F, F], f32, isOutput=False)  # W^T
    w_ext = nc.declare_dram_parameter("w", [F, F], f32, isOutput=False)    # W
    a1_ext = nc.declare_dram_parameter("a1", [F, 1], f32, isOutput=False)
    a2_ext = nc.declare_dram_parameter("a2", [F, 1], f32, isOutput=False)
    out_ext = nc.declare_dram_parameter("out", [RPC, F], f32, isOutput=True)

    with tile.TileContext(nc) as tc, ExitStack() as ctx:
        const = ctx.enter_context(tc.tile_pool(name="const", bufs=1))
        setup = ctx.enter_context(tc.tile_pool(name="setup", bufs=3))
        psum = ctx.enter_context(tc.tile_pool(name="psum", bufs=8, space="PSUM"))
        adj_pool = ctx.enter_context(tc.tile_pool(name="adjp", bufs=8))
        work = ctx.enter_context(tc.tile_pool(name="work", bufs=6))
        pexp = ctx.enter_context(tc.tile_pool(name="pexp", bufs=6))
        outp = ctx.enter_context(tc.tile_pool(name="outp", bufs=2))

        wt_sb = const.tile([F, F], f32)
        nc.sync.dma_start(out=wt_sb, in_=wt_ext[:, :])
        w_sb = const.tile([F, F], f32)
        nc.sync.dma_start(out=w_sb, in_=w_ext[:, :])
        a1_sb = const.tile([F, 1], f32)
        nc.sync.dma_start(out=a1_sb, in_=a1_ext[:, :])
        a2_sb = const.tile([F, 1], f32)
        nc.sync.dma_start(out=a2_sb, in_=a2_ext[:, :])
        ones_row = const.tile([1, 128], f32)
        nc.vector.memset(ones_row, 1.0)

        # persistent tensors
        whext_t = [const.tile([128, 8, F + 1], bf16, name=f"whext{_}")
                   for _ in range(8)]
        sj_cols = [const.tile([128, 8], f32, name=f"sjc{_}")
                   for _ in range(8)]            # s2, column layout, per k
        sibc = const.tile([128, RPC], f32)        # s1 own rows, bcast over parts
        sibc_bf = const.tile([128, RPC], bf16)    # bf16 twin for the DVE path
        for jj in range(8):
            nc.vector.memset(whext_t[jj][:, :, F:F + 1], 1.0)

        # w1 = W^T a1, w2 = W^T a2 ; wt2 = [W^T | w2]
        ps_w = psum.tile([128, 512], f32, tag="acc")
        nc.tensor.matmul(ps_w[:, 0:1], lhsT=w_sb, rhs=a1_sb, start=True, stop=True)
        nc.tensor.matmul(ps_w[:, 1:2], lhsT=w_sb, rhs=a2_sb, start=True, stop=True)
        w1c = const.tile([128, 1], f32)
        nc.vector.tensor_copy(out=w1c, in_=ps_w[:, 0:1])
        wt2_sb = const.tile([F, F + 1], f32)
        nc.vector.tensor_copy(out=wt2_sb[:, 0:F], in_=wt_sb)
        nc.vector.tensor_copy(out=wt2_sb[:, F:F + 1], in_=ps_w[:, 1:2])

        # s1 own rows -> free layout -> broadcast across partitions
        si_sb = const.tile([1, RPC], f32)
        for kk in range(RPC // 512):
            hTlc = setup.tile([128, 512], f32, tag="hTlc", name=f"hTlc{kk}")
            nc.sync.dma_start(out=hTlc,
                              in_=hTloc_ext[:, 512 * kk:512 * kk + 512])
            ps_si = psum.tile([128, 512], f32, tag="acc", name=f"psi{kk}")
            nc.tensor.matmul(ps_si[0:1, 0:512], lhsT=w1c, rhs=hTlc,
                             start=True, stop=True)
            nc.vector.tensor_copy(out=si_sb[0:1, 512 * kk:512 * kk + 512],
                                  in_=ps_si[0:1, 0:512])
        for kk in range(RPC // 512):
            ps_sib = psum.tile([128, 512], f32, tag="acc", name=f"psib{kk}")
            nc.tensor.matmul(ps_sib[:, 0:512], lhsT=ones_row,
                             rhs=si_sb[0:1, 512 * kk:512 * kk + 512],
                             start=True, stop=True)
            nc.scalar.copy(out=sibc[:, 512 * kk:512 * kk + 512],
                           in_=ps_sib[:, 0:512])
            nc.vector.tensor_copy(out=sibc_bf[:, 512 * kk:512 * kk + 512],
                                  in_=ps_sib[:, 0:512])

        # stream hT chunks: whext tiles (Wh | 1) and s2 columns via [W^T | w2]
        def emit_setup_k(k):
            hTc = setup.tile([128, 1024], f32, tag="hTc", name=f"hTc{k}")
            nc.sync.dma_start(out=hTc, in_=hT_ext[:, 1024 * k:1024 * k + 1024])
            for m in range(8):
                ci = 8 * k + m
                ps2 = psum.tile([128, 512], f32, tag="acc", name=f"pwh{ci}")
                nc.tensor.matmul(ps2[:, 0:F + 1],
                                 lhsT=hTc[:, 128 * m:128 * m + 128],
                                 rhs=wt2_sb, start=True, stop=True)
                nc.vector.tensor_copy(out=whext_t[k][:, m, 0:F],
                                      in_=ps2[:, 0:F])
                nc.vector.tensor_copy(out=sj_cols[k][:, m:m + 1],
                                      in_=ps2[:, F:F + 1])

        # main chunk: 128 source nodes x all own rows
        def emit_main_ci(ci, accs, pair_buf):
            adjT = adj_pool.tile([128, RPC], bf16, tag="adjT", name=f"adjT{ci}")
            nc.sync.dma_start(out=adjT,
                              in_=adj_ext[128 * ci:128 * ci + 128, :])
            q = ci % 4
            if (((ci + 1) * PHI) // NCI) > ((ci * PHI) // NCI):
                # DVE path: 4x-ts add, 2x-tt mask, 4x-ts scale, 2x-tt max
                u1 = work.tile([128, RPC], bf16, tag="u1", name=f"u1_{ci}")
                nc.vector.tensor_scalar(
                    out=u1, in0=sibc_bf,
                    scalar1=sj_cols[ci // 8][:, ci % 8:ci % 8 + 1],
                    scalar2=None, op0=AL.add)
                T0 = work.tile([128, RPC], bf16, tag="T0", name=f"T0_{ci}")
                nc.vector.tensor_tensor(out=T0, in0=u1, in1=adjT, op=AL.mult)
                u2 = work.tile([128, RPC], bf16, tag="u2", name=f"u2_{ci}")
                nc.vector.tensor_scalar(out=u2, in0=T0, scalar1=NEG_SLOPE,
                                        scalar2=None, op0=AL.mult)
                nc.vector.tensor_tensor(out=pair_buf[:, RPC * q:RPC * q + RPC],
                                        in0=u2, in1=T0, op=AL.max)
            else:
                L = work.tile([128, RPC], bf16, tag="L", name=f"L_{ci}")
                nc.scalar.activation(out=L, in_=sibc, func=AF.Prelu,
                                     bias=sj_cols[ci // 8][:, ci % 8:ci % 8 + 1],
                                     alpha=NEG_SLOPE)
                nc.vector.tensor_tensor(out=pair_buf[:, RPC * q:RPC * q + RPC],
                                        in0=L, in1=adjT, op=AL.mult)
            if q == 3:
                P2 = pexp.tile([128, 4 * RPC], bf16, tag="P", name=f"P{ci}")
                nc.scalar.activation(out=P2, in_=pair_buf, func=AF.Exp)
                for h in range(4):
                    cih = ci - 3 + h
                    for t in range(RT):
                        nc.tensor.matmul(
                            accs[t],
                            lhsT=P2[:, RPC * h + 128 * t:RPC * h + 128 * t + 128],
                            rhs=whext_t[cih // 8][:, cih % 8, :],
                            start=(cih == 0 and t % 2 == 0),
                            stop=(cih == NCI - 1),
                            skip_group_check=True)

        acc_banks = [psum.tile([128, 512], f32, tag="acc", name=f"accb{b}")
                     for b in range(RT // 2)]
        accs = [acc_banks[t // 2][:, 256 * (t % 2):256 * (t % 2) + F + 1]
                for t in range(RT)]

        def emit_main(ci_iter, pair):
            if ci_iter % 4 == 0:
                pair = work.tile([128, 4 * RPC], bf16, tag="T",
                                 name=f"Tp{ci_iter}")
            emit_main_ci(ci_iter, accs, pair)
            return pair

        pair = None
        emit_setup_k(0)
        emit_setup_k(1)
        ci_iter = 0
        for k in range(2, 8):
            emit_setup_k(k)
            while ci_iter < 8 * (k - 1):
                pair = emit_main(ci_iter, pair)
                ci_iter += 1
        while ci_iter < NCI:
            pair = emit_main(ci_iter, pair)
            ci_iter += 1

        for t in range(RT):
            rinv = outp.tile([128, 1], f32, tag="rinv", name=f"rinv{t}")
            nc.vector.reciprocal(rinv, accs[t][:, F:F + 1])
            o_t = outp.tile([128, F], f32, tag="o", name=f"o{t}")
            nc.vector.tensor_scalar(out=o_t, in0=accs[t][:, 0:F],
                                    scalar1=rinv[:, 0:1], scalar2=None,
                                    op0=AL.mult)
            nc.sync.dma_start(out=out_ext[128 * t:128 * t + 128, :], in_=o_t)

    nc.compile()
    return nc


def _get_nc():
    if "nc" not in _CACHE:
        _CACHE["nc"] = _build()
    return _CACHE["nc"]


def kernel(h, adj, W, a, _trace=False, _trace_kwargs=None):
    h = np.ascontiguousarray(np.asarray(h, dtype=np.float32))
    adj = np.asarray(adj, dtype=np.float32)
    W = np.asarray(W, dtype=np.float32)
    a = np.asarray(a, dtype=np.float32)

    wt = np.ascontiguousarray(W.T)                    # [fi, fo]
    a1c = np.ascontiguousarray(a[0, :F].reshape(F, 1))
    a2c = np.ascontiguousarray(a[0, F:].reshape(F, 1))
    hT = np.ascontiguousarray(h.T)                    # [fi, n]
    adjT_bf = adj.astype(ml_dtypes.bfloat16).T        # 0/1 values: lossless

    nc = _get_nc()
    in_maps = []
    for c in range(NCORES):
        r0 = c * RPC
        in_maps.append({
            "adjT": np.ascontiguousarray(adjT_bf[:, r0:r0 + RPC]),
            "hT": hT,
            "hT_loc": np.ascontiguousarray(hT[:, r0:r0 + RPC]),
            "wt": wt,
            "w": W,
            "a1": a1c,
            "a2": a2c,
        })
    kw = {}
    if _trace:
        kw["trace"] = True
        kw.update(_trace_kwargs or {})
    res = run_bass_kernel_spmd(nc, in_maps, core_ids=list(range(NCORES)), **kw)
    out = np.concatenate([res.results[c]["out"] for c in range(NCORES)], axis=0)
    if _trace:
        return out, res
    return out
